# revision 1
# baseline (speedup 1.0000x reference)
"""Distance-aware masking kernel for Trainium2 (8 NeuronCores).

Computes mask[i,j,:] = W2 @ relu(W1 @ [r_i - c_j, |r_i - c_j|] + b1) + b2
for N=4096 nodes, DIM_OUT=8, sharded by rows across 8 cores.

Strategy (per core, 512 rows):
  - All pairwise terms that are linear in (row-features x col-features) are
    computed on the TensorEngine as small-K matmuls with host-precomputed
    basis operands (split into bf16 triples for fp32-grade accuracy):
      S~[p,j] = a_m^2 * (|r_i|^2 - 2 r_i.c_j + |c_j|^2 + eps)   (squared dist)
      V~[p,j] = alpha_m(i) - g_m(j)                             (linear MLP part)
    with partitions packed p = 4*di + m (32 rows x {3 hidden units + pad}).
  - ACT: D = sqrt(S~) = |a_m| * dist;  DVE: u = D*sign(a_m) + V~ (fused
    scalar_tensor_tensor), h = relu(u) -> float32r.
  - TensorEngine mixes 3 hidden units -> 8 outputs with a block-diagonal
    W2 matmul (f32r), output partitions q = 8*di2 + o.
  - PSUM -> SBUF copy (DVE/ACT), DMA to DRAM scratch laid out [i*8+o, j];
    host transposes to [i, j, o], patches the exact diagonal, concatenates.
"""

import sys

sys.path.insert(0, "/opt/trn_rl_repo")

import numpy as np
import ml_dtypes

N = 4096
N_CORES = 8
ROWS = N // N_CORES          # 512 rows per core
IB = 32                      # i-rows per block (x4 slots = 128 partitions)
NB = ROWS // IB              # 16 blocks
J = 512                      # j-tile (columns per tile)
NJ = N // J                  # 8 j-tiles
EPS = 3e-5                   # dist^2 floor; protects sqrt from f32 cancellation
DIM = 3
DIM_OUT = 8

_BF = ml_dtypes.bfloat16

_CACHE = {}


def _split3(x):
    hi = x.astype(_BF)
    r = x - hi.astype(np.float32)
    mid = r.astype(_BF)
    lo = (r - mid.astype(np.float32)).astype(_BF)
    return hi, mid, lo


def _split2(x):
    hi = x.astype(_BF)
    lo = (x - hi.astype(np.float32)).astype(_BF)
    return hi, lo


def _build_program():
    """Build + compile the SPMD Bass program once (shapes are static)."""
    import concourse.bass as bass  # noqa: F401
    import concourse.mybir as mybir
    import concourse.tile as tile
    from concourse import bacc

    nc = bacc.Bacc("TRN2", target_bir_lowering=False, num_devices=N_CORES)

    f32 = mybir.dt.float32
    f16 = mybir.dt.float16
    bf16 = mybir.dt.bfloat16

    sv_lhsT = nc.dram_tensor("sv_lhsT", [46, NB * 128], bf16, kind="ExternalInput").ap()
    sv_rhs = nc.dram_tensor("sv_rhs", [46, 2 * N], bf16, kind="ExternalInput").ap()
    mixw = nc.dram_tensor("mixw", [128, 128], f16, kind="ExternalInput").ap()
    sgn = nc.dram_tensor("sgn", [128, 1], f32, kind="ExternalInput").ap()
    scratch = nc.dram_tensor("scratch", [N, N], f32, kind="ExternalOutput").ap()

    with tile.TileContext(nc) as tc:
        with tc.tile_pool(name="const", bufs=1) as cp, \
             tc.tile_pool(name="work", bufs=4) as wp, \
             tc.tile_pool(name="outp", bufs=6) as op, \
             tc.tile_pool(name="psv", bufs=2, space="PSUM") as psv, \
             tc.tile_pool(name="psm", bufs=2, space="PSUM") as psm:

            t_sv_lhsT = cp.tile([46, NB * 128], bf16, tag="t_sv_lhsT")
            nc.sync.dma_start(t_sv_lhsT[:], sv_lhsT)
            t_sv_rhs = cp.tile([46, 2 * N], bf16, tag="t_sv_rhs")
            nc.sync.dma_start(t_sv_rhs[:], sv_rhs)
            t_mixw = cp.tile([128, 128], f16, tag="t_mixw")
            nc.sync.dma_start(t_mixw[:], mixw)
            t_sgn = cp.tile([128, 1], f32, tag="t_sgn")
            nc.sync.dma_start(t_sgn[:], sgn)

            for b in range(NB):
                lcol = slice(b * 128, b * 128 + 128)
                for jt in range(NJ):
                    jcol = slice(jt * J, (jt + 1) * J)

                    ps_sv = psv.tile([128, 2 * J], f32, tag="ps_sv")
                    for half in range(2):
                        nc.tensor.matmul(
                            ps_sv[:, half * J:(half + 1) * J],
                            t_sv_lhsT[:, lcol],
                            t_sv_rhs[:, (2 * jt + half) * J:(2 * jt + half + 1) * J],
                            start=True, stop=True,
                        )

                    t_d = wp.tile([128, J], f32, tag="t_d")
                    nc.scalar.activation(
                        t_d[:], ps_sv[:, 0:J], mybir.ActivationFunctionType.Sqrt
                    )
                    t_u = wp.tile([128, J], f32, tag="t_u")
                    nc.vector.scalar_tensor_tensor(
                        t_u[:], t_d[:], t_sgn[:], ps_sv[:, J:2 * J],
                        mybir.AluOpType.mult, mybir.AluOpType.add,
                    )
                    t_h = wp.tile([128, J], f16, tag="t_h")
                    nc.vector.tensor_scalar_max(t_h[:], t_u[:], 0.0)

                    t_o = op.tile([128, 2 * J], f32, tag="t_o")
                    ps_o = psm.tile([128, 2 * J], f32, tag="ps_o")
                    for w in range(2):
                        pr = slice(64 * w, 64 * w + 64)
                        nc.tensor.matmul(
                            ps_o[:, w * J:(w + 1) * J], t_mixw[pr, :], t_h[pr, :],
                            start=True, stop=True,
                        )
                    if (b * NJ + jt) % 4 < 3:
                        nc.scalar.copy(t_o[:], ps_o[:])
                    else:
                        nc.vector.tensor_copy(t_o[:], ps_o[:])
                    # scratch rows r = 256*b + 128*w + q  <->  sbuf [q, w*J+j]
                    row0 = b * IB * DIM_OUT
                    dview = scratch[row0:row0 + 256, jcol].rearrange(
                        "(w q) j -> q w j", w=2
                    )
                    nc.sync.dma_start(dview, t_o[:].rearrange("q (w j) -> q w j", w=2))

    nc.compile()
    return nc


def _host_inputs(node_coords, W1, b1, W2, b2):
    """Build per-core input maps (all small host-side numpy work)."""
    coords = node_coords.astype(np.float32)
    W1 = W1.astype(np.float32)
    b1 = b1.astype(np.float32)
    W2 = W2.astype(np.float32)
    b2 = b2.astype(np.float32)

    a = W1[:, 3]                       # [3] dist coefficients
    a2 = a * a
    Wc = W1[:, :3]                     # [3,3] coord coefficients
    g = coords @ Wc.T                  # [N,3]  g_m(j)
    c2 = (coords * coords).sum(1)      # [N]

    # ---- shared rhs bases ----
    s_base_r = np.zeros((5, N), np.float32)
    s_base_r[0:3] = coords.T
    s_base_r[3] = c2
    s_base_r[4] = 1.0

    v_base_r = np.zeros((4, N), np.float32)
    v_base_r[0] = 1.0
    v_base_r[1:4] = g.T

    Rh, Rm, Rl = _split3(s_base_r)
    vRh, vRl = _split2(v_base_r)

    # ---- mix weights (block-diagonal W2), duplicated for both windows ----
    mixw = np.zeros((128, 128), np.float32)
    for w in range(2):
        for di in range(16):
            for m in range(3):
                mixw[64 * w + 4 * di + m, 8 * di + 0:8 * di + 8] = W2[:, m]
    # rows are p = 4*di+m (K side), cols are q = 8*di+o (M side)

    sgn = np.zeros((128, 1), np.float32)
    for m in range(3):
        sgn[m::4, 0] = np.sign(a[m])

    in_maps = []
    for c in range(N_CORES):
        r = coords[c * ROWS:(c + 1) * ROWS]          # [512,3]
        r2 = (r * r).sum(1)                          # [512]
        alpha = r @ Wc.T + b1                        # [512,3]

        # packed column index for local row i (0..511): b*128 + 4*(i%32) + m
        i_idx = np.arange(ROWS)
        col = (i_idx // IB) * 128 + 4 * (i_idx % IB)  # [512] base col (m=0)

        s_base_l = np.zeros((5, NB * 128), np.float32)
        v_base_l = np.zeros((4, NB * 128), np.float32)
        for m in range(3):
            cm = col + m
            s_base_l[0:3, cm] = (-2.0 * a2[m]) * r.T
            s_base_l[3, cm] = a2[m]
            s_base_l[4, cm] = a2[m] * (r2 + EPS)
            v_base_l[0, cm] = alpha[:, m]
            v_base_l[m + 1, cm] = -1.0

        Lh, Lm, Ll = _split3(s_base_l)
        vLh, vLl = _split2(v_base_l)

        # pair order: big (hh) terms first so cancellation happens early
        s_lhsT = np.vstack([Lh, Lh, Lm, Lh, Ll, Lm])      # [30, 2048]
        s_rhs = np.vstack([Rh, Rm, Rh, Rl, Rh, Rm])       # [30, 4096]
        v_lhsT = np.vstack([vLh, vLh, vLl, vLl])          # [16, 2048]
        v_rhs = np.vstack([vRh, vRl, vRh, vRl])           # [16, 4096]

        # merged S|V operands: one K=46 matmul per (block, jtile) computes
        # S in columns [0:J] and V in columns [J:2J] of the psum tile
        sv_lhsT = np.vstack([s_lhsT, v_lhsT])             # [46, 2048]
        sv_rhs = np.zeros((46, 2 * N), _BF)
        sv_view = sv_rhs.reshape(46, NJ, 2, J)
        sv_view[0:30, :, 0, :] = s_rhs.reshape(30, NJ, J)
        sv_view[30:46, :, 1, :] = v_rhs.reshape(16, NJ, J)

        in_maps.append({
            "sv_lhsT": np.ascontiguousarray(sv_lhsT),
            "sv_rhs": np.ascontiguousarray(sv_rhs),
            "mixw": mixw.astype(np.float16),
            "sgn": sgn,
        })
    return in_maps


def kernel(node_coords, W1, b1, W2, b2):
    from concourse.bass_utils import run_bass_kernel_spmd

    if "nc" not in _CACHE:
        _CACHE["nc"] = _build_program()
    nc = _CACHE["nc"]

    in_maps = _host_inputs(node_coords, W1, b1, W2, b2)
    res = run_bass_kernel_spmd(nc, in_maps, core_ids=list(range(N_CORES)))
    _CACHE["last_res"] = res

    out = np.empty((N, N, DIM_OUT), np.float32)
    for c in range(N_CORES):
        sc = res.results[c]["scratch"]                   # [4096, 4096] f16
        blk = sc.reshape(ROWS, DIM_OUT, N).transpose(0, 2, 1)
        out[c * ROWS:(c + 1) * ROWS] = blk

    # b2 is handled here (the device mix omits it)
    if np.any(b2):
        out += b2.astype(np.float32)

    # exact diagonal (pairwise features are exactly zero there; the device
    # path has an eps floor under the sqrt, so patch on host)
    h_diag = np.maximum(b1.astype(np.float32), 0.0)
    diag = W2.astype(np.float32) @ h_diag + b2.astype(np.float32)
    idx = np.arange(N)
    out[idx, idx, :] = diag

    return out



# revision 13
# speedup vs baseline: 1.7764x; 1.7764x over previous
"""Distance-aware masking kernel for Trainium2 (8 NeuronCores), v2.

Computes mask[i,j,:] = W2 @ relu(W1 @ [r_i - c_j, |r_i - c_j|] + b1) + b2
for N=4096 nodes, DIM_OUT=8, rows sharded across 8 cores (512 rows/core).

Dataflow per core (all matmuls run in 4x-row-tiled mode, K<=32, with
tile_position pinning each matmul to one 32-partition strip):

  1. D2 matmul (K=30, per 128-row group g, per j-tile): dist^2(i,j)+eps via
     3-split bf16 basis products, psum [128 i, 512 j].
  2. ACT sqrt: d = sqrt(D2) -> SBUF f16 tile t_d [128 i, 512 j]
     (amortized: one sqrt per 4 compute iterations).
  3. Per 32-row block b = 4g+c and j-tile:
     U matmul (K=32, strip c):  ps_u[4di+m, j]  = a_m * d(32b+di, j)
     V matmul (K=3, strip c+2): ps_u          += -g_m(j)      (accumulate)
  4. relu with per-partition bias (DVE tensor_scalar):
     h = max(ps_u + alpha_m(i), 0) -> f16
  5. Mix: 4 quarter-matmuls (K=32, strips 0..3, accumulated in pairs):
     ps_o[8di'+o, w*512+j] = sum_m W2[o,m] h[...]
  6. psum -> SBUF f16 copies split between ACT and DVE (the bottleneck:
     every output value crosses ACT/DVE once; DMA cannot read PSUM).
  7. Per-block DMA of t_o [128, 2*4096] f16 to DRAM scratch rows 8*i+o.

Host: builds basis operands, assembles [N,N,8] f32 output, adds b2,
patches the exact diagonal.
"""

import sys

sys.path.insert(0, "/opt/trn_rl_repo")

import numpy as np
import ml_dtypes

N = 4096
N_CORES = 8
ROWS = N // N_CORES          # 512 rows per core
NG = 4                       # 128-row groups per core
NB = 16                      # 32-row blocks per core
J = 512                      # j-tile width
NJ = N // J                  # 8 j-tiles
EPS = 3e-5
DIM_OUT = 8

# relu runs on ACT for RELU_ACT_NUM of every RELU_ACT_DEN iterations
# (balances the fixed w-split of the ps_o copies: ACT w0, DVE w1).
RELU_ACT_NUM = 5
RELU_ACT_DEN = 12

_BF = ml_dtypes.bfloat16
_F16 = np.float16

_CACHE = {}


def _split3(x):
    hi = x.astype(_BF)
    r = x - hi.astype(np.float32)
    mid = r.astype(_BF)
    lo = (r - mid.astype(np.float32)).astype(_BF)
    return hi, mid, lo


def _build_program():
    import concourse.bass as bass  # noqa: F401
    import concourse.mybir as mybir
    import concourse.tile as tile
    from concourse import bacc

    nc = bacc.Bacc("TRN2", target_bir_lowering=False, num_devices=N_CORES)

    f32 = mybir.dt.float32
    f16 = mybir.dt.float16
    bf16 = mybir.dt.bfloat16

    rd2 = nc.dram_tensor("rd2", [128, N], bf16, kind="ExternalInput").ap()
    ld2 = nc.dram_tensor("ld2", [128, 4 * 128], bf16, kind="ExternalInput").ap()
    g3 = nc.dram_tensor("g3", [128, N], f16, kind="ExternalInput").ap()
    lv = nc.dram_tensor("lv", [128, 128], f16, kind="ExternalInput").ap()
    pmat = nc.dram_tensor("pmat", [128, 4 * 128], f16, kind="ExternalInput").ap()
    mixw = nc.dram_tensor("mixw", [128, 128], f16, kind="ExternalInput").ap()
    alpha = nc.dram_tensor("alpha", [128, NB], f32, kind="ExternalInput").ap()
    scratch = nc.dram_tensor("scratch", [ROWS * DIM_OUT, N], f16,
                             kind="ExternalOutput").ap()

    SQRT = mybir.ActivationFunctionType.Sqrt
    COPYF = mybir.ActivationFunctionType.Copy
    RELUF = mybir.ActivationFunctionType.Relu
    ADD = mybir.AluOpType.add
    MAX = mybir.AluOpType.max

    with tile.TileContext(nc) as tc:
        with tc.tile_pool(name="const", bufs=1) as cp, \
             tc.tile_pool(name="dpool", bufs=24) as dp, \
             tc.tile_pool(name="hpool", bufs=4) as hp, \
             tc.tile_pool(name="opool", bufs=2) as op, \
             tc.tile_pool(name="psd", bufs=2, space="PSUM") as psd, \
             tc.tile_pool(name="psu", bufs=2, space="PSUM") as psu, \
             tc.tile_pool(name="pso", bufs=2, space="PSUM") as pso:

            t_rd2 = cp.tile([128, N], bf16, tag="t_rd2")
            nc.sync.dma_start(t_rd2[:], rd2)
            t_ld2 = cp.tile([128, 4 * 128], bf16, tag="t_ld2")
            nc.sync.dma_start(t_ld2[:], ld2)
            t_g3 = cp.tile([128, N], f16, tag="t_g3")
            nc.sync.dma_start(t_g3[:], g3)
            t_lv = cp.tile([128, 128], f16, tag="t_lv")
            nc.sync.dma_start(t_lv[:], lv)
            t_p = cp.tile([128, 4 * 128], f16, tag="t_p")
            nc.sync.dma_start(t_p[:], pmat)
            t_mixw = cp.tile([128, 128], f16, tag="t_mixw")
            nc.sync.dma_start(t_mixw[:], mixw)
            t_alpha = cp.tile([128, NB], f32, tag="t_alpha")
            nc.sync.dma_start(t_alpha[:], alpha)

            t_d = {}

            def emit_d2(g, jt):
                ps = psd.tile([128, J], f32, tag="ps_d2")
                nc.tensor.matmul(
                    ps[:], t_ld2[:, 128 * g:128 * (g + 1)],
                    t_rd2[:, jt * J:(jt + 1) * J],
                    start=True, stop=True,
                )
                td = dp.tile([128, J], f16, tag="t_d")
                nc.scalar.activation(td[:], ps[:], SQRT)
                t_d[(g, jt)] = td

            def emit_mix_copy(ctx):
                th, b, jt, t_o = ctx
                ps_o = pso.tile([128, 2 * J], f32, tag="ps_o")
                for w in range(2):
                    nc.tensor.matmul(
                        ps_o[:, w * J:(w + 1) * J],
                        t_mixw[64 * w:64 * w + 64, :],
                        th[64 * w:64 * w + 64, :],
                        start=True, stop=True,
                    )
                # split copy: ACT drains w0, DVE drains w1 (2D contiguous)
                nc.scalar.activation(
                    t_o[:, jt * J:(jt + 1) * J], ps_o[:, 0:J], COPYF
                )
                nc.vector.tensor_copy(
                    t_o[:, N + jt * J:N + (jt + 1) * J], ps_o[:, J:2 * J]
                )

            for jt in range(NJ):
                emit_d2(0, jt)

            pending = None
            t_o = None
            k_iter = 0
            for g in range(NG):
                for c in range(4):
                    b = 4 * g + c
                    s = c
                    for jt in range(NJ):
                        if jt == 0:
                            t_o = op.tile([128, 2 * N], f16, tag="t_o")
                        td = t_d[(g, jt)]
                        ps_u = psu.tile([128, J], f32, tag="ps_u")
                        nc.tensor.matmul(
                            ps_u[:], t_p[:, 128 * c:128 * (c + 1)], td[:],
                            start=True, stop=False,
                        )
                        nc.tensor.matmul(
                            ps_u[:], t_lv[:, :],
                            t_g3[:, jt * J:(jt + 1) * J],
                            start=False, stop=True,
                        )
                        if pending is not None:
                            emit_mix_copy(pending)
                            if pending[2] == NJ - 1:  # finished block's t_o
                                pb = pending[1]
                                dview = scratch[256 * pb:256 * pb + 256, :] \
                                    .rearrange("(w q) j -> q w j", w=2)
                                nc.sync.dma_start(
                                    dview,
                                    pending[3][:].rearrange(
                                        "q (w j) -> q w j", w=2),
                                )
                        th = hp.tile([128, J], f16, tag="t_h")
                        if k_iter % RELU_ACT_DEN < RELU_ACT_NUM:
                            nc.scalar.activation(
                                th[:], ps_u[:], RELUF,
                                bias=t_alpha[:, b:b + 1],
                            )
                        else:
                            nc.vector.tensor_scalar(
                                th[:], ps_u[:], t_alpha[:, b:b + 1], 0.0,
                                op0=ADD, op1=MAX,
                            )
                        k_iter += 1
                        pending = (th, b, jt, t_o)
                        # prefetch next group's d tiles mid-way through
                        if c == 2 and g + 1 < NG:
                            emit_d2(g + 1, jt)

            emit_mix_copy(pending)
            dview = scratch[256 * 15:256 * 15 + 256, :] \
                .rearrange("(w q) j -> q w j", w=2)
            nc.sync.dma_start(
                dview, pending[3][:].rearrange("q (w j) -> q w j", w=2)
            )

    nc.compile()
    return nc


def _host_inputs(node_coords, W1, b1, W2, b2):
    coords = node_coords.astype(np.float32)
    W1 = W1.astype(np.float32)
    b1 = b1.astype(np.float32)
    W2 = W2.astype(np.float32)

    a = W1[:, 3]
    Wc = W1[:, :3]
    g = coords @ Wc.T                  # [N, 3]
    c2 = (coords * coords).sum(1)

    # ---- D2 rhs bases (shared across cores): R = [-2c, 1, c2], 3-split ----
    R = np.zeros((5, N), np.float32)
    R[0:3] = -2.0 * coords.T
    R[3] = 1.0
    R[4] = c2
    Rh, Rm, Rl = _split3(R)
    rd2 = np.zeros((128, N), _BF)
    rd2[0:30] = np.vstack([Rh, Rm, Rh, Rl, Rh, Rm])    # [30, N]

    # ---- g3 rows 0-2 (f16) ----
    g3 = np.zeros((128, N), _F16)
    g3[0:3] = g.T.astype(_F16)

    # ---- lhsT_v: lv[m', 4di+m] = -1 if m'==m ----
    lv = np.zeros((128, 128), _F16)
    for di in range(32):
        for m in range(3):
            lv[m, 4 * di + m] = -1.0

    # ---- P variant per block-in-group c: P[k, 128c + 4di+m] = a[m]
    #      iff k == 32c + di ----
    pmat = np.zeros((128, 4 * 128), _F16)
    for c in range(4):
        for di in range(32):
            for m in range(3):
                pmat[32 * c + di, 128 * c + 4 * di + m] = a[m]

    # ---- mix weights ----
    mixw = np.zeros((128, 128), np.float32)
    for p in range(128):
        di, m = p // 4, p % 4
        if m == 3:
            continue
        q_di = di if di < 16 else di - 16
        mixw[p, 8 * q_di:8 * q_di + 8] = W2[:, m]
    mixw = mixw.astype(_F16)

    in_maps = []
    for core in range(N_CORES):
        r = coords[core * ROWS:(core + 1) * ROWS]
        r2 = (r * r).sum(1)
        alpha_full = r @ Wc.T + b1                 # [512, 3]

        L = np.zeros((5, ROWS), np.float32)
        L[0:3] = r.T
        L[3] = r2 + EPS
        L[4] = 1.0
        Lh, Lm, Ll = _split3(L)
        lstack = np.vstack([Lh, Lh, Lm, Lh, Ll, Lm])   # [30, 512]
        ld2 = np.zeros((128, 4 * 128), _BF)
        for gi in range(NG):
            ld2[0:30, 128 * gi:128 * (gi + 1)] = \
                lstack[:, gi * 128:(gi + 1) * 128]

        t_alpha = np.zeros((128, NB), np.float32)
        i_idx = np.arange(32)
        for b in range(NB):
            blk = alpha_full[32 * b:32 * b + 32]       # [32, 3]
            for m in range(3):
                t_alpha[4 * i_idx + m, b] = blk[:, m]

        in_maps.append({
            "rd2": rd2, "ld2": ld2, "g3": g3, "lv": lv,
            "pmat": pmat, "mixw": mixw, "alpha": t_alpha,
        })
    return in_maps


def kernel(node_coords, W1, b1, W2, b2):
    from concourse.bass_utils import run_bass_kernel_spmd

    if "nc" not in _CACHE:
        _CACHE["nc"] = _build_program()
    nc = _CACHE["nc"]

    in_maps = _host_inputs(node_coords, W1, b1, W2, b2)
    res = run_bass_kernel_spmd(nc, in_maps, core_ids=list(range(N_CORES)))
    _CACHE["last_res"] = res

    out = np.empty((N, N, DIM_OUT), np.float32)
    for c in range(N_CORES):
        sc = res.results[c]["scratch"]             # [4096, 4096] f16
        blk = sc.reshape(ROWS, DIM_OUT, N).transpose(0, 2, 1)
        out[c * ROWS:(c + 1) * ROWS] = blk

    b2f = b2.astype(np.float32)
    if np.any(b2f):
        out += b2f

    h_diag = np.maximum(b1.astype(np.float32), 0.0)
    diag = W2.astype(np.float32) @ h_diag + b2f
    idx = np.arange(N)
    out[idx, idx, :] = diag

    return out
